# revision 33
# baseline (speedup 1.0000x reference)
"""Windowed multi-head attention with relative-position bias on 8 TRN2 NeuronCores.

Data-parallel over batch: each of the 8 cores processes 4 of the 32 batch
elements end-to-end (QKV projection -> biased softmax attention -> output
projection); weights and the (precomputed, exponentiated) bias table are
replicated. No collectives are needed; the host shards inputs and
concatenates the per-core outputs.

Layout strategy (per core, b_local=4):
  - qkv is computed TRANSPOSED (q^T,k^T in [dh, n] layout) so that
    S^T = k^T.T @ q^T comes out in [j, i] layout with partition=j, which is
    exactly what the P@V matmul wants as its moving operand.
  - S^T matmuls have K=32, so 4 heads are packed per 128-partition tile and
    issued as bursts to 4 distinct PE row-groups (tile_position=(32h, 0)) ->
    they run concurrently on the systolic array (~3x measured on TRN2).
  - P@V matmuls have M=33 (32 outputs + a ones-column for the softmax
    denominator); heads alternate PE col-groups (tile_position=(0,0)/(0,64),
    each in its OWN psum bank) so neighbouring PVs overlap too.
  - softmax skips max-subtraction (scores are tiny: |s| < ~1) and the bias
    is applied as a multiplicative exp(bias) table, split across the vector
    and gpsimd engines.
  - the output projection consumes O^T = [d, n] directly and produces
    y^T = [d_out, n], which is already the required (b, d, H, W) layout.
  - matmul operands are bf16 (full PE rate); accumulation stays fp32.
"""

import numpy as np
import ml_dtypes

import concourse.bass as bass
import concourse.mybir as mybir
import concourse.tile as tile
from concourse import bacc
from concourse.bass_utils import run_bass_kernel_spmd

# problem shape (hardcoded; kernel.py must be self-contained)
B, D, WIN = 32, 256, 25
N = WIN * WIN            # 625 tokens
P = 128
NPAD = 640               # 5 j-tiles of 128
H, DH = 8, 32            # heads x head_dim
NCORES = 8
BL = B // NCORES         # 4 batch elements per core
KT_D = D // P            # 2 contraction tiles over d
NJT = NPAD // P          # 5 j-tiles
ICH = [(0, 320), (320, 305)]  # i-chunks (psum bank holds 512 fp32)
G = DH + 1               # 33: head group stride in V (32 outputs + ones col)
HP = 3                   # heads per 128-partition tile in O^T (3*33=99)
OPT = 3                  # O^T partition tiles (3+3+2 heads)
NSTR = 632               # padded i-stride (x2B = 4-byte aligned slices for DVE 2x)

F32 = mybir.dt.float32
BF16 = mybir.dt.bfloat16
EXP = mybir.ActivationFunctionType.Exp
NBF = ml_dtypes.bfloat16


def build_nc():
    # Bacc (not raw Bass): its compile pass legalizes multi-wait matmuls
    # (move_matmul_waits_to_ldweights / generate_event_semaphores), which
    # walrus codegen requires.
    nc = bacc.Bacc()
    x_d = nc.dram_tensor("x", (BL, D, N), BF16, kind="ExternalInput")
    wqk_d = nc.dram_tensor("w_qk", (D, 2 * D), BF16, kind="ExternalInput")
    wv_d = nc.dram_tensor("w_v", (D, D), BF16, kind="ExternalInput")
    wo_d = nc.dram_tensor("w_o", (OPT * P, D), BF16, kind="ExternalInput")
    eb_d = nc.dram_tensor("expb", (H, NPAD, N), BF16, kind="ExternalInput")
    out_d = nc.dram_tensor("out", (BL, D, N), F32, kind="ExternalOutput")

    with tile.TileContext(nc) as tc:
        with (
            tc.tile_pool(name="consts", bufs=1) as consts,
            tc.tile_pool(name="persist", bufs=1) as persist,
            tc.tile_pool(name="ebs", bufs=2) as ebs,
            tc.tile_pool(name="es", bufs=24) as es,
            tc.tile_pool(name="pvs", bufs=2) as pvs,
            tc.tile_pool(name="bcs", bufs=2) as bcs,
            tc.tile_pool(name="ys", bufs=2) as ys,
            tc.tile_pool(name="ps", bufs=2, space="PSUM") as ps,
        ):
            # ---------------- inputs (replicated weights, all-batch x) ----------------
            wqk = consts.tile([P, KT_D, 2 * D], BF16)
            nc.sync.dma_start(wqk[:], wqk_d.rearrange("(kt p) m -> p kt m", p=P))
            wv = consts.tile([P, KT_D, D], BF16)
            nc.sync.dma_start(wv[:], wv_d.rearrange("(kt p) m -> p kt m", p=P))
            wo = consts.tile([P, OPT, D], BF16)
            nc.sync.dma_start(wo[:], wo_d.rearrange("(kt p) m -> p kt m", p=P))
            xall = consts.tile([P, BL, KT_D, N], BF16)
            nc.sync.dma_start(xall[:], x_d.rearrange("b (kt p) i -> p b kt i", p=P))

            qkT, V, OT, den = {}, {}, {}, {}

            # ---------------- stage 1: projections, per batch element ----------------
            for b in range(BL):
                # q^T (free tiles 0,1 = W cols 0..255) and k^T (tiles 2,3),
                # 4 heads per 128-partition tile at offsets 0/32/64/96
                t_qkT = persist.tile([P, 4, NPAD], BF16, tag=f"qkT{b}", name=f"qkT{b}")
                nc.gpsimd.memset(t_qkT[:, 2:4, N:NPAD], 0.0)  # zero k^T j-pad
                for mt in range(4):
                    for c0, cw in ICH:
                        acc = ps.tile([P, 512], F32, tag="ps", name="acc", bufs=8)
                        for kt in range(KT_D):
                            nc.tensor.matmul(
                                acc[:, :cw],
                                wqk[:, kt, mt * P : (mt + 1) * P],
                                xall[:, b, kt, c0 : c0 + cw],
                                start=(kt == 0),
                                stop=(kt == KT_D - 1),
                            )
                        nc.vector.tensor_copy(out=t_qkT[:, mt, c0 : c0 + cw], in_=acc[:, :cw])

                # V in [j, head-grouped d] layout, with a ones column per head
                t_V = persist.tile([P, NJT, H * G], BF16, tag=f"V{b}", name=f"V{b}")
                nc.gpsimd.memset(t_V[:], 1.0)
                for jt in range(NJT):
                    jr = min(P, N - jt * P)  # 128,128,128,128,113
                    acc = ps.tile([P, 512], F32, tag="ps", name="acc", bufs=8)
                    for kt in range(KT_D):
                        nc.tensor.matmul(
                            acc[:jr, :D],
                            xall[:, b, kt, jt * P : jt * P + jr],
                            wv[:, kt, :],
                            start=(kt == 0),
                            stop=(kt == KT_D - 1),
                        )
                    nc.vector.tensor_copy(
                        out=t_V[:jr, jt].rearrange("p (h g) -> p h g", g=G)[:, :, :DH],
                        in_=acc[:jr, :D].rearrange("p (h g) -> p h g", g=DH),
                    )

                # O^T staging: head h lives at rows (h%3)*33.. of ptile h//3
                t_OT = persist.tile([P, OPT, NSTR], BF16, tag=f"OT{b}", name=f"OT{b}")
                nc.gpsimd.memset(t_OT[:], 0.0)  # pad rows must be 0, not NaN garbage
                t_den = persist.tile([H, N], BF16, tag=f"den{b}", name=f"den{b}")
                qkT[b], V[b], OT[b], den[b] = t_qkT, t_V, t_OT, t_den

            # ---------------- stage 2: attention ----------------
            # head groups of 4 (one q/k partition tile). S psum tiles span 2
            # banks (512+113 chunks) so exp + bias-mult run once per (head,
            # j-tile) at full 625 width; P@V processes 2 heads at a time (4
            # psum banks), col-alternating so neighbouring PVs overlap.
            ebg = {}
            for g in range(H // 4):
                ebg[g] = ebs.tile([P, 4, NJT, NSTR], BF16, tag="ebg", name=f"ebg{g}")
                nc.sync.dma_start(
                    ebg[g][:, :, :, :N],
                    eb_d[4 * g : 4 * (g + 1)].rearrange("h (jt p) i -> p h jt i", p=P),
                )
            for g in range(H // 4):
                for b in range(BL):
                    pvSg = pvs.tile([G, 4, NSTR], BF16, tag="pvS", name=f"pvS{g}{b}")
                    for ci, (c0, cw) in enumerate(ICH):
                        E2 = {}
                        for jt in range(NJT):
                            for hq in range(4):
                                off = hq * DH
                                st = ps.tile([P, 512], F32, tag="ps", name="st", bufs=8)
                                nc.tensor.matmul(
                                    st[:, :cw],
                                    qkT[b][off : off + DH, 2 + g, jt * P : (jt + 1) * P],
                                    qkT[b][off : off + DH, g, c0 : c0 + cw],
                                    tile_position=(off, 0),
                                )
                                E = es.tile([P, 512], BF16, tag="E", name=f"E{hq}")
                                nc.scalar.activation(E[:, :cw], st[:, :cw], EXP)
                                nc.vector.tensor_mul(
                                    out=E[:, :cw],
                                    in0=E[:, :cw],
                                    in1=ebg[g][:, hq, jt, c0 : c0 + cw],
                                )
                                E2[jt, hq] = E
                        pvt = [
                            ps.tile([P, 512], F32, tag="ps", name=f"pv{hq}", bufs=8)
                            for hq in range(4)
                        ]
                        for jt in range(NJT):
                            for hq in range(4):
                                h = 4 * g + hq
                                row = (hq % 2) * 64
                                nc.tensor.matmul(
                                    pvt[hq][row : row + G, :cw],
                                    V[b][:, jt, h * G : (h + 1) * G],
                                    E2[jt, hq][:, :cw],
                                    start=(jt == 0),
                                    stop=(jt == NJT - 1),
                                    tile_position=(0, row),
                                )
                        for hq in range(4):
                            row = (hq % 2) * 64
                            nc.scalar.copy(
                                pvSg[:, hq, c0 : c0 + cw],
                                pvt[hq][row : row + G, :cw],
                            )
                    for hq in range(4):
                        h = 4 * g + hq
                        pt, slot = divmod(h, HP)
                        nc.sync.dma_start(
                            OT[b][slot * G : slot * G + DH, pt, :N], pvSg[:DH, hq, :N]
                        )
                    nc.sync.dma_start(
                        den[b][4 * g : 4 * (g + 1), :], pvSg[DH : DH + 1, :, :N]
                    )
            # stage 3 trails the whole program: lowest scheduler priority, so
            # its ops backfill engine gaps instead of preempting attention
            for b in range(BL):
                _stage3(nc, b, OT, den, wo, bcs, ys, ps, out_d)

    return nc


def _stage3(nc, b, OT, den, wo, bcs, ys, ps, out_d):
    """Normalize O^T by the softmax denominators and apply W_out."""
    denf = bcs.tile([H, N], F32, tag="denf", name=f"denf{b}")
    nc.vector.tensor_copy(out=denf[:], in_=den[b][:])
    scr = bcs.tile([H, N], F32, tag="scr", name=f"scr{b}")
    nc.vector.reciprocal_approx_accurate(out=denf[:], in_=denf[:], scratch=scr[:])
    denb = bcs.tile([H, NSTR], BF16, tag="denb", name=f"denb{b}")
    nc.vector.tensor_copy(out=denb[:, :N], in_=denf[:])
    bc = bcs.tile([P, OPT, NSTR], BF16, tag="bc", name=f"bc{b}")
    for pt in range(OPT):
        nh = min(HP, H - pt * HP)  # 3,3,2
        nc.sync.dma_start(
            bc[: nh * G, pt, :N],
            denb[pt * HP : pt * HP + nh, None, :N].to_broadcast((nh, G, N)),
        )
    for pt in range(OPT):
        nh = min(HP, H - pt * HP)
        for c0, cw in ICH:
            nc.vector.tensor_mul(
                out=OT[b][: nh * G, pt, c0 : c0 + cw],
                in0=OT[b][: nh * G, pt, c0 : c0 + cw],
                in1=bc[: nh * G, pt, c0 : c0 + cw],
            )
    yb = ys.tile([P, KT_D, N], F32, tag="yb", name=f"yb{b}")
    for mt in range(KT_D):
        for c0, cw in ICH:
            yp = ps.tile([P, 512], F32, tag="ps", name="yp", bufs=8)
            for kt in range(OPT):
                nc.tensor.matmul(
                    yp[:, :cw],
                    wo[:, kt, mt * P : (mt + 1) * P],
                    OT[b][:, kt, c0 : c0 + cw],
                    start=(kt == 0),
                    stop=(kt == OPT - 1),
                )
            nc.vector.tensor_copy(out=yb[:, mt, c0 : c0 + cw], in_=yp[:, :cw])
    nc.sync.dma_start(out_d[b].rearrange("(mt p) i -> p mt i", p=P), yb[:])


def _host_prep(W_qkv, W_out, rel_emb):
    scale = DH ** -0.5
    wqk = np.ascontiguousarray(W_qkv[:, : 2 * D]).copy()
    wqk[:, :D] *= scale  # fold q scaling into the weights
    wv = np.ascontiguousarray(W_qkv[:, 2 * D :])
    # W_out rows rearranged into the packed O^T layout; denominator/pad rows zero
    wo = np.zeros((OPT * P, D), np.float32)
    for h in range(H):
        pt, slot = divmod(h, HP)
        wo[pt * P + slot * G : pt * P + slot * G + DH] = W_out[h * DH : (h + 1) * DH]
    # relative-position bias -> exp(bias)^T, padded along j to 640 with zeros
    pos = np.arange(WIN)
    gi, gj = np.meshgrid(pos, pos, indexing="ij")
    grid = np.stack([gi.reshape(-1), gj.reshape(-1)], -1)
    rel = grid[:, None, :] - grid[None, :, :] + (WIN - 1)
    idx = rel[..., 0] * (2 * WIN - 1) + rel[..., 1]  # [i, j]
    eb = np.zeros((H, NPAD, N), np.float32)
    eb[:, :N, :] = np.exp(rel_emb[idx]).transpose(2, 1, 0)  # -> [h, j, i]
    return wqk.astype(NBF), wv.astype(NBF), wo.astype(NBF), eb.astype(NBF)


def _install_ntff_hook():
    """This image lacks antenv.axon_hooks; shim it and register the ctypes
    NTFF profiling hook so trace=True yields exec_time_ns. Bench-only."""
    import sys
    import types

    if "antenv.axon_hooks" not in sys.modules:
        mod = types.ModuleType("antenv.axon_hooks")
        mod._hook = None
        mod.set_axon_ntff_profile_hook = lambda h: setattr(mod, "_hook", h)
        mod.get_axon_ntff_profile_hook = lambda: mod._hook
        sys.modules["antenv.axon_hooks"] = mod
    try:
        from trn_agent_boot.trn_boot import _ntff_profile_via_ctypes

        hook = _ntff_profile_via_ctypes("/opt/axon/libaxon_pjrt.so")
        sys.modules["antenv.axon_hooks"].set_axon_ntff_profile_hook(hook)
    except Exception as e:  # degrade to untimed run
        print(f"NTFF hook install failed ({e}); running without trace")


def _enable_ldw_opt():
    """Let walrus dedupe LDWEIGHTS for back-to-back matmuls that share a
    stationary operand (~800 weight reloads in this kernel). Correctness is
    re-checked against the reference after every compile."""
    from concourse import bass_utils

    if getattr(bass_utils, "_ldw_patched", False):
        return
    orig = bass_utils.run_command

    def patched(argv, **kwargs):
        argv = [a.replace("--enable-ldw-opt=false", "--enable-ldw-opt=true") for a in argv]
        return orig(argv, **kwargs)

    bass_utils.run_command = patched
    bass_utils._ldw_patched = True


def kernel(x, W_qkv, W_out, rel_emb, _bench=False):
    x = np.ascontiguousarray(
        np.asarray(x, np.float32).reshape(B, D, N).astype(NBF)
    )
    wqk, wv, wo, eb = _host_prep(
        np.asarray(W_qkv, np.float32),
        np.asarray(W_out, np.float32),
        np.asarray(rel_emb, np.float32),
    )
    nc = build_nc()
    nc.finalize()
    in_maps = [
        {"x": x[c * BL : (c + 1) * BL], "w_qk": wqk, "w_v": wv, "w_o": wo, "expb": eb}
        for c in range(NCORES)
    ]
    if _bench:
        _install_ntff_hook()
    res = run_bass_kernel_spmd(nc, in_maps, core_ids=list(range(NCORES)), trace=_bench)
    if _bench:
        kernel._last = res
    out = np.concatenate([np.asarray(res.results[c]["out"]) for c in range(NCORES)], axis=0)
    return out.reshape(B, D, WIN, WIN).astype(np.float32)


# revision 34
# speedup vs baseline: 1.1332x; 1.1332x over previous
"""Windowed multi-head attention with relative-position bias on 8 TRN2 NeuronCores.

Data-parallel over batch: each of the 8 cores processes 4 of the 32 batch
elements end-to-end (QKV projection -> biased softmax attention -> output
projection); weights and the (precomputed, exponentiated) bias table are
replicated. No collectives are needed; the host shards inputs and
concatenates the per-core outputs.

Layout strategy (per core, b_local=4):
  - qkv is computed TRANSPOSED (q^T,k^T in [dh, n] layout) so that
    S^T = k^T.T @ q^T comes out in [j, i] layout with partition=j, which is
    exactly what the P@V matmul wants as its moving operand.
  - S^T matmuls have K=32, so 4 heads are packed per 128-partition tile and
    issued as bursts to 4 distinct PE row-groups (tile_position=(32h, 0)) ->
    they run concurrently on the systolic array (~3x measured on TRN2).
  - P@V matmuls have M=33 (32 outputs + a ones-column for the softmax
    denominator); heads alternate PE col-groups (tile_position=(0,0)/(0,64),
    each in its OWN psum bank) so neighbouring PVs overlap too.
  - softmax skips max-subtraction (scores are tiny: |s| < ~1) and the bias
    is applied as a multiplicative exp(bias) table, split across the vector
    and gpsimd engines.
  - the output projection consumes O^T = [d, n] directly and produces
    y^T = [d_out, n], which is already the required (b, d, H, W) layout.
  - matmul operands are bf16 (full PE rate); accumulation stays fp32.
"""

import numpy as np
import ml_dtypes

import concourse.bass as bass
import concourse.mybir as mybir
import concourse.tile as tile
from concourse import bacc
from concourse.bass_utils import run_bass_kernel_spmd

# problem shape (hardcoded; kernel.py must be self-contained)
B, D, WIN = 32, 256, 25
N = WIN * WIN            # 625 tokens
P = 128
NPAD = 640               # 5 j-tiles of 128
H, DH = 8, 32            # heads x head_dim
NCORES = 8
BL = B // NCORES         # 4 batch elements per core
KT_D = D // P            # 2 contraction tiles over d
NJT = NPAD // P          # 5 j-tiles
ICH = [(0, 320), (320, 305)]  # i-chunks (psum bank holds 512 fp32)
G = DH + 1               # 33: head group stride in V (32 outputs + ones col)
HP = 3                   # heads per 128-partition tile in O^T (3*33=99)
OPT = 3                  # O^T partition tiles (3+3+2 heads)
NSTR = 632               # padded i-stride (x2B = 4-byte aligned slices for DVE 2x)

F32 = mybir.dt.float32
BF16 = mybir.dt.bfloat16
EXP = mybir.ActivationFunctionType.Exp
NBF = ml_dtypes.bfloat16


def build_nc():
    # Bacc (not raw Bass): its compile pass legalizes multi-wait matmuls
    # (move_matmul_waits_to_ldweights / generate_event_semaphores), which
    # walrus codegen requires.
    nc = bacc.Bacc()
    x_d = nc.dram_tensor("x", (BL, D, N), BF16, kind="ExternalInput")
    wqk_d = nc.dram_tensor("w_qk", (D, 2 * D), BF16, kind="ExternalInput")
    wv_d = nc.dram_tensor("w_v", (D, D), BF16, kind="ExternalInput")
    wo_d = nc.dram_tensor("w_o", (OPT * P, D), BF16, kind="ExternalInput")
    eb_d = nc.dram_tensor("expb", (H, NPAD, N), BF16, kind="ExternalInput")
    out_d = nc.dram_tensor("out", (BL, D, N), F32, kind="ExternalOutput")

    with tile.TileContext(nc) as tc:
        with (
            tc.tile_pool(name="consts", bufs=1) as consts,
            tc.tile_pool(name="persist", bufs=1) as persist,
            tc.tile_pool(name="ebs", bufs=2) as ebs,
            tc.tile_pool(name="es", bufs=24) as es,
            tc.tile_pool(name="pvs", bufs=2) as pvs,
            tc.tile_pool(name="bcs", bufs=2) as bcs,
            tc.tile_pool(name="ys", bufs=2) as ys,
            tc.tile_pool(name="ps", bufs=2, space="PSUM") as ps,
        ):
            # ---------------- inputs (replicated weights, all-batch x) ----------------
            wqk = consts.tile([P, KT_D, 2 * D], BF16)
            nc.sync.dma_start(wqk[:], wqk_d.rearrange("(kt p) m -> p kt m", p=P))
            wv = consts.tile([P, KT_D, D], BF16)
            nc.sync.dma_start(wv[:], wv_d.rearrange("(kt p) m -> p kt m", p=P))
            wo = consts.tile([P, OPT, D], BF16)
            nc.sync.dma_start(wo[:], wo_d.rearrange("(kt p) m -> p kt m", p=P))
            xall = consts.tile([P, BL, KT_D, N], BF16)
            nc.sync.dma_start(xall[:], x_d.rearrange("b (kt p) i -> p b kt i", p=P))

            qkT, V, OT, den = {}, {}, {}, {}

            # ---------------- stage 1: projections, per batch element ----------------
            for b in range(BL):
                # q^T (free tiles 0,1 = W cols 0..255) and k^T (tiles 2,3),
                # 4 heads per 128-partition tile at offsets 0/32/64/96
                t_qkT = persist.tile([P, 4, NPAD], BF16, tag=f"qkT{b}", name=f"qkT{b}")
                nc.gpsimd.memset(t_qkT[:, 2:4, N:NPAD], 0.0)  # zero k^T j-pad
                for mt in range(4):
                    for c0, cw in ICH:
                        acc = ps.tile([P, 512], F32, tag="ps", name="acc", bufs=8)
                        for kt in range(KT_D):
                            nc.tensor.matmul(
                                acc[:, :cw],
                                wqk[:, kt, mt * P : (mt + 1) * P],
                                xall[:, b, kt, c0 : c0 + cw],
                                start=(kt == 0),
                                stop=(kt == KT_D - 1),
                            )
                        nc.vector.tensor_copy(out=t_qkT[:, mt, c0 : c0 + cw], in_=acc[:, :cw])

                # V in [j, head-grouped d] layout, with a ones column per head
                t_V = persist.tile([P, NJT, H * G], BF16, tag=f"V{b}", name=f"V{b}")
                nc.gpsimd.memset(t_V[:], 1.0)
                for jt in range(NJT):
                    jr = min(P, N - jt * P)  # 128,128,128,128,113
                    acc = ps.tile([P, 512], F32, tag="ps", name="acc", bufs=8)
                    for kt in range(KT_D):
                        nc.tensor.matmul(
                            acc[:jr, :D],
                            xall[:, b, kt, jt * P : jt * P + jr],
                            wv[:, kt, :],
                            start=(kt == 0),
                            stop=(kt == KT_D - 1),
                        )
                    nc.vector.tensor_copy(
                        out=t_V[:jr, jt].rearrange("p (h g) -> p h g", g=G)[:, :, :DH],
                        in_=acc[:jr, :D].rearrange("p (h g) -> p h g", g=DH),
                    )

                # O^T staging: head h lives at rows (h%3)*33.. of ptile h//3
                t_OT = persist.tile([P, OPT, NSTR], BF16, tag=f"OT{b}", name=f"OT{b}")
                nc.gpsimd.memset(t_OT[:], 0.0)  # pad rows must be 0, not NaN garbage
                t_den = persist.tile([H, N], BF16, tag=f"den{b}", name=f"den{b}")
                qkT[b], V[b], OT[b], den[b] = t_qkT, t_V, t_OT, t_den

            # ---------------- stage 2: attention ----------------
            # head groups of 4 (one q/k partition tile). S psum tiles span 2
            # banks (512+113 chunks) so exp + bias-mult run once per (head,
            # j-tile) at full 625 width; P@V processes 2 heads at a time (4
            # psum banks), col-alternating so neighbouring PVs overlap.
            ebg = {}
            for g in range(H // 4):
                ebg[g] = ebs.tile([P, 4, NJT, NSTR], BF16, tag="ebg", name=f"ebg{g}")
                nc.sync.dma_start(
                    ebg[g][:, :, :, :N],
                    eb_d[4 * g : 4 * (g + 1)].rearrange("h (jt p) i -> p h jt i", p=P),
                )
            for g in range(H // 4):
                for b in range(BL):
                    pvSg = pvs.tile([G, 4, NSTR], BF16, tag="pvS", name=f"pvS{g}{b}")
                    for ci, (c0, cw) in enumerate(ICH):
                        E2 = {}
                        for jt in range(NJT):
                            for hq in range(4):
                                off = hq * DH
                                st = ps.tile([P, 512], F32, tag="ps", name="st", bufs=8)
                                nc.tensor.matmul(
                                    st[:, :cw],
                                    qkT[b][off : off + DH, 2 + g, jt * P : (jt + 1) * P],
                                    qkT[b][off : off + DH, g, c0 : c0 + cw],
                                    tile_position=(off, 0),
                                )
                                E = es.tile([P, 512], BF16, tag="E", name=f"E{hq}")
                                nc.scalar.activation(E[:, :cw], st[:, :cw], EXP)
                                nc.vector.tensor_mul(
                                    out=E[:, :cw],
                                    in0=E[:, :cw],
                                    in1=ebg[g][:, hq, jt, c0 : c0 + cw],
                                )
                                E2[jt, hq] = E
                        pvt = [
                            ps.tile([P, 512], F32, tag="ps", name=f"pv{hq}", bufs=8)
                            for hq in range(4)
                        ]
                        for jt in range(NJT):
                            for hq in range(4):
                                h = 4 * g + hq
                                row = (hq % 2) * 64
                                nc.tensor.matmul(
                                    pvt[hq][row : row + G, :cw],
                                    V[b][:, jt, h * G : (h + 1) * G],
                                    E2[jt, hq][:, :cw],
                                    start=(jt == 0),
                                    stop=(jt == NJT - 1),
                                    tile_position=(0, row),
                                )
                        for hq in range(4):
                            row = (hq % 2) * 64
                            nc.vector.tensor_copy(
                                out=pvSg[:, hq, c0 : c0 + cw],
                                in_=pvt[hq][row : row + G, :cw],
                            )
                    for hq in range(4):
                        h = 4 * g + hq
                        pt, slot = divmod(h, HP)
                        nc.sync.dma_start(
                            OT[b][slot * G : slot * G + DH, pt, :N], pvSg[:DH, hq, :N]
                        )
                    nc.sync.dma_start(
                        den[b][4 * g : 4 * (g + 1), :], pvSg[DH : DH + 1, :, :N]
                    )
            # stage 3 trails the whole program: lowest scheduler priority, so
            # its ops backfill engine gaps instead of preempting attention
            for b in range(BL):
                _stage3(nc, b, OT, den, wo, bcs, ys, ps, out_d)

    return nc


def _stage3(nc, b, OT, den, wo, bcs, ys, ps, out_d):
    """Normalize O^T by the softmax denominators and apply W_out."""
    denf = bcs.tile([H, N], F32, tag="denf", name=f"denf{b}")
    nc.vector.tensor_copy(out=denf[:], in_=den[b][:])
    scr = bcs.tile([H, N], F32, tag="scr", name=f"scr{b}")
    nc.vector.reciprocal_approx_accurate(out=denf[:], in_=denf[:], scratch=scr[:])
    denb = bcs.tile([H, NSTR], BF16, tag="denb", name=f"denb{b}")
    nc.vector.tensor_copy(out=denb[:, :N], in_=denf[:])
    bc = bcs.tile([P, OPT, NSTR], BF16, tag="bc", name=f"bc{b}")
    for pt in range(OPT):
        nh = min(HP, H - pt * HP)  # 3,3,2
        nc.sync.dma_start(
            bc[: nh * G, pt, :N],
            denb[pt * HP : pt * HP + nh, None, :N].to_broadcast((nh, G, N)),
        )
    for pt in range(OPT):
        nh = min(HP, H - pt * HP)
        for c0, cw in ICH:
            nc.vector.tensor_mul(
                out=OT[b][: nh * G, pt, c0 : c0 + cw],
                in0=OT[b][: nh * G, pt, c0 : c0 + cw],
                in1=bc[: nh * G, pt, c0 : c0 + cw],
            )
    yb = ys.tile([P, KT_D, N], F32, tag="yb", name=f"yb{b}")
    for mt in range(KT_D):
        for c0, cw in ICH:
            yp = ps.tile([P, 512], F32, tag="ps", name="yp", bufs=8)
            for kt in range(OPT):
                nc.tensor.matmul(
                    yp[:, :cw],
                    wo[:, kt, mt * P : (mt + 1) * P],
                    OT[b][:, kt, c0 : c0 + cw],
                    start=(kt == 0),
                    stop=(kt == OPT - 1),
                )
            nc.scalar.copy(yb[:, mt, c0 : c0 + cw], yp[:, :cw])
    nc.sync.dma_start(out_d[b].rearrange("(mt p) i -> p mt i", p=P), yb[:])


def _host_prep(W_qkv, W_out, rel_emb):
    scale = DH ** -0.5
    wqk = np.ascontiguousarray(W_qkv[:, : 2 * D]).copy()
    wqk[:, :D] *= scale  # fold q scaling into the weights
    wv = np.ascontiguousarray(W_qkv[:, 2 * D :])
    # W_out rows rearranged into the packed O^T layout; denominator/pad rows zero
    wo = np.zeros((OPT * P, D), np.float32)
    for h in range(H):
        pt, slot = divmod(h, HP)
        wo[pt * P + slot * G : pt * P + slot * G + DH] = W_out[h * DH : (h + 1) * DH]
    # relative-position bias -> exp(bias)^T, padded along j to 640 with zeros
    pos = np.arange(WIN)
    gi, gj = np.meshgrid(pos, pos, indexing="ij")
    grid = np.stack([gi.reshape(-1), gj.reshape(-1)], -1)
    rel = grid[:, None, :] - grid[None, :, :] + (WIN - 1)
    idx = rel[..., 0] * (2 * WIN - 1) + rel[..., 1]  # [i, j]
    eb = np.zeros((H, NPAD, N), np.float32)
    eb[:, :N, :] = np.exp(rel_emb[idx]).transpose(2, 1, 0)  # -> [h, j, i]
    return wqk.astype(NBF), wv.astype(NBF), wo.astype(NBF), eb.astype(NBF)


def _install_ntff_hook():
    """This image lacks antenv.axon_hooks; shim it and register the ctypes
    NTFF profiling hook so trace=True yields exec_time_ns. Bench-only."""
    import sys
    import types

    if "antenv.axon_hooks" not in sys.modules:
        mod = types.ModuleType("antenv.axon_hooks")
        mod._hook = None
        mod.set_axon_ntff_profile_hook = lambda h: setattr(mod, "_hook", h)
        mod.get_axon_ntff_profile_hook = lambda: mod._hook
        sys.modules["antenv.axon_hooks"] = mod
    try:
        from trn_agent_boot.trn_boot import _ntff_profile_via_ctypes

        hook = _ntff_profile_via_ctypes("/opt/axon/libaxon_pjrt.so")
        sys.modules["antenv.axon_hooks"].set_axon_ntff_profile_hook(hook)
    except Exception as e:  # degrade to untimed run
        print(f"NTFF hook install failed ({e}); running without trace")


def _enable_ldw_opt():
    """Let walrus dedupe LDWEIGHTS for back-to-back matmuls that share a
    stationary operand (~800 weight reloads in this kernel). Correctness is
    re-checked against the reference after every compile."""
    from concourse import bass_utils

    if getattr(bass_utils, "_ldw_patched", False):
        return
    orig = bass_utils.run_command

    def patched(argv, **kwargs):
        argv = [a.replace("--enable-ldw-opt=false", "--enable-ldw-opt=true") for a in argv]
        return orig(argv, **kwargs)

    bass_utils.run_command = patched
    bass_utils._ldw_patched = True


def kernel(x, W_qkv, W_out, rel_emb, _bench=False):
    x = np.ascontiguousarray(
        np.asarray(x, np.float32).reshape(B, D, N).astype(NBF)
    )
    wqk, wv, wo, eb = _host_prep(
        np.asarray(W_qkv, np.float32),
        np.asarray(W_out, np.float32),
        np.asarray(rel_emb, np.float32),
    )
    nc = build_nc()
    nc.finalize()
    in_maps = [
        {"x": x[c * BL : (c + 1) * BL], "w_qk": wqk, "w_v": wv, "w_o": wo, "expb": eb}
        for c in range(NCORES)
    ]
    if _bench:
        _install_ntff_hook()
    res = run_bass_kernel_spmd(nc, in_maps, core_ids=list(range(NCORES)), trace=_bench)
    if _bench:
        kernel._last = res
    out = np.concatenate([np.asarray(res.results[c]["out"]) for c in range(NCORES)], axis=0)
    return out.reshape(B, D, WIN, WIN).astype(np.float32)
